# revision 26
# baseline (speedup 1.0000x reference)
"""Multi-head self-attention (B=4, S=2048, E=1024, H=16, causal) on 8 NeuronCores.

Sharding (Megatron-style, per hint): data-parallel over B (4) x tensor-parallel
over heads (2 groups of 8 heads). Core c handles batch c//2 with head-group
c%2: Wq/Wk/Wv sharded column-wise, Wo row-wise. Each core emits a partial
out-projection [S, E]; the host sums each pair of partials (the "all-reduce")
and adds bo.

Per-core kernel (all matmuls bf16, fp32 accumulation):
  - host supplies x[b].T so Q^T,K^T ([d,s]) and V ([s,d]) come straight off
    the projections with no on-chip transposes
  - scores computed transposed (S^T = K Q^T, [keys, queries]) with causal
    block-skipping; the two heads of a pair issue as row-tiled (K=64)
    matmuls that can run concurrently; exp on ScalarE with fused 1/sqrt(D)
    scale reads PSUM directly and writes bf16; diagonal-block masking runs
    on GpSimd to keep the DVE queue clear
  - softmax denominator comes free from a ones-column appended to V in the
    attn @ V matmul; pav is evacuated to SBUF immediately (frees the PSUM
    bank) and normalization is deferred to end-of-qc and batched:
    denominators gathered at 32-aligned partitions, one free-dim-bound
    reciprocal per 4 heads, col-tiled rank-1 PE broadcasts, one vector
    multiply per head
  - V-bias is folded into V at projection time (softmax weights sum to 1)
  - a junk-matmul warmup burst spans the input-DMA window so the HAM clock
    gate is at 8/8 when the first projection lands
"""

import numpy as np
import ml_dtypes

B, S, E, H, D = 4, 2048, 1024, 16, 64
HPC = 8          # heads per core
DC = HPC * D     # 512 sharded feature cols per core
EC = E // 128    # 8 e-chunks
TT = S // 128    # 16 token tiles
QCH = S // 512   # 4 query chunks
NB = S // 128    # 16 key blocks

BF16 = ml_dtypes.bfloat16

_CACHE = {}


def _build():
    import concourse.tile as tile
    from concourse import bacc, mybir

    F32 = mybir.dt.float32
    BF = mybir.dt.bfloat16
    AF = mybir.ActivationFunctionType
    ALU = mybir.AluOpType

    nc = bacc.Bacc("TRN2", target_bir_lowering=False, debug=False, num_devices=8)

    xT_d = nc.dram_tensor("xT", [EC, 128, S], BF, kind="ExternalInput")
    wq_d = nc.dram_tensor("wq", [EC, 128, DC], BF, kind="ExternalInput")
    wk_d = nc.dram_tensor("wk", [EC, 128, DC], BF, kind="ExternalInput")
    wv_d = nc.dram_tensor("wv", [EC, 128, DC], BF, kind="ExternalInput")
    wo_d = nc.dram_tensor("wo", [DC // 128, 128, E], BF, kind="ExternalInput")
    bq_d = nc.dram_tensor("bq", [128, 4], F32, kind="ExternalInput")
    bk_d = nc.dram_tensor("bk", [128, 4], F32, kind="ExternalInput")
    bvb_d = nc.dram_tensor("bvb", [128, HPC, 64], BF, kind="ExternalInput")
    mask_d = nc.dram_tensor("mask", [128, 128], BF, kind="ExternalInput")
    out_d = nc.dram_tensor("out", [TT, 128, E], F32, kind="ExternalOutput")

    with tile.TileContext(nc) as tc:
        with tc.tile_pool(name="const", bufs=1) as cp, \
             tc.tile_pool(name="expp", bufs=1) as expp, \
             tc.tile_pool(name="work", bufs=3) as wp, \
             tc.tile_pool(name="unp", bufs=1) as unp, \
             tc.tile_pool(name="ps_s", bufs=2, space="PSUM") as ps_s, \
             tc.tile_pool(name="ps_av", bufs=2, space="PSUM") as ps_av, \
             tc.tile_pool(name="ps_w", bufs=2, space="PSUM") as ps_w:

            # ---- persistent SBUF tensors + input DMAs ----
            xT = [cp.tile([128, S], BF, tag=f"xT{k}", name=f"xT{k}") for k in range(EC)]
            wq = [cp.tile([128, DC], BF, tag=f"wq{k}", name=f"wq{k}") for k in range(EC)]
            wk = [cp.tile([128, DC], BF, tag=f"wk{k}", name=f"wk{k}") for k in range(EC)]
            wv = [cp.tile([128, DC], BF, tag=f"wv{k}", name=f"wv{k}") for k in range(EC)]
            wo = [cp.tile([128, E], BF, tag=f"wo{k}", name=f"wo{k}") for k in range(DC // 128)]
            # spread input DMAs across the 3 DMA-capable queues
            for k in range(EC):
                nc.sync.dma_start(xT[k][:], xT_d.ap()[k])
                nc.scalar.dma_start(wq[k][:], wq_d.ap()[k])
                nc.gpsimd.dma_start(wk[k][:], wk_d.ap()[k])
                nc.scalar.dma_start(wv[k][:], wv_d.ap()[k])
            for k in range(DC // 128):
                nc.gpsimd.dma_start(wo[k][:], wo_d.ap()[k])
            bq = cp.tile([128, 4], F32, tag="bq", name="bq")
            bk = cp.tile([128, 4], F32, tag="bk", name="bk")
            bvb = cp.tile([128, HPC, 64], BF, tag="bvb", name="bvb")
            mask = cp.tile([128, 128], BF, tag="mask", name="mask")
            nc.gpsimd.dma_start(bq[:], bq_d.ap())
            nc.gpsimd.dma_start(bk[:], bk_d.ap())
            nc.gpsimd.dma_start(bvb[:], bvb_d.ap())
            nc.gpsimd.dma_start(mask[:], mask_d.ap())
            ones = cp.tile([128, 64], BF, tag="ones", name="ones")
            nc.any.memset(ones[:], 1.0)
            # softmax-denominator batch tiles: heads at partitions 0/32/64/96
            # (DVE partition bases must be 32-aligned); junk rows preset to
            # 1.0 so the batched reciprocal stays finite
            den = [cp.tile([128, 512], BF, tag=f"den{j}", name=f"den{j}")
                   for j in range(2)]
            rcb = [cp.tile([128, 512], BF, tag=f"rcb{j}", name=f"rcb{j}")
                   for j in range(2)]
            nc.any.memset(den[0][:], 1.0)
            nc.any.memset(den[1][:], 1.0)
            # HAM warmup: the PE is idle for the first ~14us (input DMA);
            # a junk-matmul burst flips the clock gate to 8/8 before the
            # first real projection lands
            wps = ps_w.tile([128, 512], F32, tag="psw", name="psw")
            for _ in range(160):
                nc.tensor.matmul(wps[0:64, 0:64], ones[:, 0:64],
                                 ones[:, 0:64], start=True, stop=True)

            QT = [cp.tile([128, S], BF, tag=f"QT{t}", name=f"QT{t}") for t in range(4)]
            KT = [cp.tile([128, S], BF, tag=f"KT{t}", name=f"KT{t}") for t in range(4)]
            V = [cp.tile([128, HPC, 66], BF, tag=f"V{s}", name=f"V{s}") for s in range(TT)]
            AOT = [cp.tile([128, S], BF, tag=f"AOT{t}", name=f"AOT{t}") for t in range(4)]

            filler = []

            def proj_group(w_sb, b_sb, dst, t, qc):
                def emit():
                    ps = ps_w.tile([128, 512], F32, tag="psw", name="psw")
                    for k in range(EC):
                        nc.tensor.matmul(
                            ps[:],
                            w_sb[k][:, t * 128:(t + 1) * 128],
                            xT[k][:, qc * 512:(qc + 1) * 512],
                            start=(k == 0), stop=(k == EC - 1))
                    nc.vector.tensor_scalar(
                        dst[t][:, qc * 512:(qc + 1) * 512], ps[:],
                        b_sb[:, t:t + 1], None, ALU.add)
                return emit

            def v_group(s):
                def emit():
                    ps = ps_w.tile([128, 512], F32, tag="psw", name="psw")
                    for k in range(EC):
                        nc.tensor.matmul(
                            ps[:],
                            xT[k][:, s * 128:(s + 1) * 128],
                            wv[k][:],
                            start=(k == 0), stop=(k == EC - 1))
                    # V-bias folded in here: softmax weights sum to 1, so
                    # adding bv to V equals adding bv to the attention output
                    nc.vector.tensor_tensor(
                        V[s][:, :, 0:64],
                        ps[:].rearrange("p (h d) -> p h d", d=64),
                        bvb[:], ALU.add)
                    nc.any.memset(V[s][:, :, 64:65], 1.0)
                return emit

            def d_group(s):
                def emit():
                    osb = wp.tile([128, E], F32, tag="osb", name="osb")
                    for n in range(2):
                        ps = ps_w.tile([128, 512], F32, tag="psw", name="psw")
                        for k in range(DC // 128):
                            nc.tensor.matmul(
                                ps[:],
                                AOT[k][:, s * 128:(s + 1) * 128],
                                wo[k][:, n * 512:(n + 1) * 512],
                                start=(k == 0), stop=(k == DC // 128 - 1))
                        nc.vector.tensor_copy(out=osb[:, n * 512:(n + 1) * 512],
                                              in_=ps[:])
                    nc.sync.dma_start(out_d.ap()[s], osb[:])
                return emit

            # up-front: pair-0 projections + first V tiles
            for t in range(4):
                for qc in range(QCH):
                    if t == 0:
                        proj_group(wq, bq, QT, t, qc)()
                        proj_group(wk, bk, KT, t, qc)()
                    else:
                        filler.append(("qkt", t, proj_group(wq, bq, QT, t, qc)))
                        filler.append(("qkt", t, proj_group(wk, bk, KT, t, qc)))
            for s in range(TT):
                if s < 4:
                    v_group(s)()
                else:
                    filler.append(("v", s, v_group(s)))

            def emit_filler_until(pred_drop):
                keep = []
                for item in filler:
                    if pred_drop(item):
                        item[2]()
                    else:
                        keep.append(item)
                filler[:] = keep

            def emit_some_filler(n):
                for _ in range(min(n, len(filler))):
                    filler.pop(0)[2]()

            # ---- attention, head-pair interleaved, qc-outer ----
            # qc=3 (the longest phase) runs first so it absorbs the
            # projection fillers; V tiles are forced in per-round just
            # before the key block that consumes them.
            un = {}
            for qc in (3, 2, 1, 0):
                nkb = 4 * qc + 4
                for hp in range(4):
                    emit_filler_until(lambda it: it[0] == "qkt" and it[1] <= hp)
                    hA, hB = 2 * hp, 2 * hp + 1
                    pav = {}
                    expT = {}
                    pav[hA] = ps_av.tile([128, 512], F32, tag="pav", name="pav")
                    pav[hB] = ps_av.tile([128, 512], F32, tag="pav", name="pav")
                    expT[hA] = expp.tile([128, NB, 512], BF, tag="expTA",
                                         name="expTA")
                    expT[hB] = expp.tile([128, NB, 512], BF, tag="expTB",
                                         name="expTB")

                    def emit_av(kbs_offs):
                        for h in (hB, hA):
                            for kb, off in kbs_offs:
                                nc.tensor.matmul(
                                    pav[h][0:65, off:512],
                                    V[kb][:, h, 0:65],
                                    expT[h][:, kb, off:512],
                                    start=(kb == 0), stop=(kb == nkb - 1))

                    for s0 in range(0, nkb, 2):
                        kbs = list(range(s0, min(s0 + 2, nkb)))
                        emit_filler_until(
                            lambda it: it[0] == "v" and it[1] <= kbs[-1])
                        pss = {h: ps_s.tile([128, 2, 512], F32, tag="pss",
                                            name="pss")
                               for h in (hA, hB)}
                        offs = {}
                        for i, kb in enumerate(kbs):
                            dj = kb - 4 * qc
                            off = 128 * dj if dj > 0 else 0
                            offs[kb] = off
                            for h, r in ((hA, 0), (hB, 64)):
                                nc.tensor.matmul(
                                    pss[h][:, i, off:512],
                                    KT[hp][r:r + 64, kb * 128:(kb + 1) * 128],
                                    QT[hp][r:r + 64,
                                           qc * 512 + off:(qc + 1) * 512],
                                    start=True, stop=True)
                        for h in (hB, hA):
                            # one full-width exp per round (junk columns
                            # below the causal off are never read by AV);
                            # emitted B-first so the first-queued score MM
                            # of the next round frees last (pairs issue
                            # together); diagonal masking on GpSimd
                            nc.scalar.activation(
                                expT[h][:, s0:s0 + len(kbs), :],
                                pss[h][:, 0:len(kbs), :],
                                AF.Exp, scale=0.125)
                            for i, kb in enumerate(kbs):
                                dj = kb - 4 * qc
                                if dj >= 0:
                                    off = offs[kb]
                                    nc.gpsimd.tensor_tensor(
                                        expT[h][:, kb, off:off + 128],
                                        expT[h][:, kb, off:off + 128],
                                        mask[:], ALU.mult)
                        emit_av([(kb, offs[kb]) for kb in kbs])
                        emit_some_filler(1)
                    # evacuate PSUM fast (one bf16 copy) so the next
                    # head-pair's AV can reuse the bank; normalization is
                    # deferred and batched at end-of-qc
                    for h, r in ((hA, 0), (hB, 64)):
                        h01 = 0 if h == hA else 1
                        i = 2 * hp + h01
                        un[i] = unp.tile([65, 512], BF, tag=f"un{i}",
                                         name=f"un{i}")
                        nc.vector.tensor_copy(out=un[i][:],
                                              in_=pav[h][0:65, :])
                # deferred batched normalization for all 8 heads of this qc
                for j in range(2):
                    for m in range(4):
                        i = 4 * j + m
                        nc.vector.tensor_copy(out=den[j][32 * m:32 * m + 1, :],
                                              in_=un[i][64:65, :])
                    with nc.allow_low_precision("softmax denom"):
                        nc.vector.reciprocal(out=rcb[j][:], in_=den[j][:])
                for i2 in range(4):
                    iA, iB = 2 * i2, 2 * i2 + 1
                    psb = ps_w.tile([128, 512], F32, tag="psw", name="psw")
                    for i, base in ((iA, 0), (iB, 64)):
                        j, m = divmod(i, 4)
                        nc.tensor.matmul(psb[base:base + 64, :],
                                         ones[32 * m:32 * m + 1, :],
                                         rcb[j][32 * m:32 * m + 1, :],
                                         start=True, stop=True,
                                         tile_position=(32 * m, base))
                    for i, base in ((iA, 0), (iB, 64)):
                        hp_i, hh = divmod(i, 2)
                        dst = AOT[hp_i][64 * hh:64 * hh + 64,
                                        qc * 512:(qc + 1) * 512]
                        nc.vector.tensor_tensor(dst, un[i][0:64, :],
                                                psb[base:base + 64, :],
                                                ALU.mult)
                # out-projection for this qc becomes filler for the next
                for s in range(qc * 4, qc * 4 + 4):
                    filler.append(("d", s, d_group(s)))
            emit_filler_until(lambda it: True)

    nc.compile()
    return nc


def _get_nc():
    if "nc" not in _CACHE:
        _CACHE["nc"] = _build()
    return _CACHE["nc"]


def _shard_inputs(x, Wq, bq, Wk, bk, Wv, bv, Wo):
    """Build the 8 per-core input maps (host-side shard/cast/transpose)."""
    x = np.asarray(x, np.float32)
    mask = np.triu(np.ones((128, 128), np.float32)).astype(BF16)  # [k, q] q>=k
    in_maps = []
    for c in range(8):
        b, hg = divmod(c, 2)
        dc = slice(hg * DC, (hg + 1) * DC)
        xT = np.ascontiguousarray(x[b].T).astype(BF16).reshape(EC, 128, S)
        wq_c = np.ascontiguousarray(Wq[:, dc]).astype(BF16).reshape(EC, 128, DC)
        wk_c = np.ascontiguousarray(Wk[:, dc]).astype(BF16).reshape(EC, 128, DC)
        wv_c = np.ascontiguousarray(Wv[:, dc]).astype(BF16).reshape(EC, 128, DC)
        wo_c = np.ascontiguousarray(Wo[dc, :]).astype(BF16).reshape(DC // 128, 128, E)
        bq_c = np.ascontiguousarray(np.asarray(bq[dc], np.float32).reshape(4, 128).T)
        bk_c = np.ascontiguousarray(np.asarray(bk[dc], np.float32).reshape(4, 128).T)
        bvb_c = np.ascontiguousarray(np.broadcast_to(
            np.asarray(bv[dc], np.float32).reshape(1, HPC, 64),
            (128, HPC, 64))).astype(BF16)
        in_maps.append({
            "xT": xT, "wq": wq_c, "wk": wk_c, "wv": wv_c, "wo": wo_c,
            "bq": bq_c, "bk": bk_c, "bvb": bvb_c, "mask": mask,
        })
    return in_maps


def kernel(x, Wq, bq, Wk, bk, Wv, bv, Wo, bo):
    from concourse.bass_utils import run_bass_kernel_spmd

    nc = _get_nc()
    in_maps = _shard_inputs(x, Wq, bq, Wk, bk, Wv, bv, Wo)
    res = run_bass_kernel_spmd(nc, in_maps, core_ids=list(range(8)))
    bo = np.asarray(bo, np.float32)
    out = np.empty((B, S, E), np.float32)
    for b in range(B):
        p0 = res.results[2 * b]["out"].reshape(S, E)
        p1 = res.results[2 * b + 1]["out"].reshape(S, E)
        out[b] = p0 + p1 + bo
    return out


# revision 27
# speedup vs baseline: 1.0033x; 1.0033x over previous
"""Multi-head self-attention (B=4, S=2048, E=1024, H=16, causal) on 8 NeuronCores.

Sharding (Megatron-style, per hint): data-parallel over B (4) x tensor-parallel
over heads (2 groups of 8 heads). Core c handles batch c//2 with head-group
c%2: Wq/Wk/Wv sharded column-wise, Wo row-wise. Each core emits a partial
out-projection [S, E]; the host sums each pair of partials (the "all-reduce")
and adds bo.

Per-core kernel (all matmuls bf16, fp32 accumulation):
  - host supplies x[b].T so Q^T,K^T ([d,s]) and V ([s,d]) come straight off
    the projections with no on-chip transposes
  - scores computed transposed (S^T = K Q^T, [keys, queries]) with causal
    block-skipping; the two heads of a pair issue as row-tiled (K=64)
    matmuls that can run concurrently; exp on ScalarE with fused 1/sqrt(D)
    scale reads PSUM directly and writes bf16; diagonal-block masking runs
    on GpSimd to keep the DVE queue clear
  - softmax denominator comes free from a ones-column appended to V in the
    attn @ V matmul; pav is evacuated to SBUF immediately (frees the PSUM
    bank) and normalization is deferred to end-of-qc and batched:
    denominators gathered at 32-aligned partitions, one free-dim-bound
    reciprocal per 4 heads, col-tiled rank-1 PE broadcasts, one vector
    multiply per head
  - V-bias is folded into V at projection time (softmax weights sum to 1)
  - a junk-matmul warmup burst spans the input-DMA window so the HAM clock
    gate is at 8/8 when the first projection lands
"""

import numpy as np
import ml_dtypes

B, S, E, H, D = 4, 2048, 1024, 16, 64
HPC = 8          # heads per core
DC = HPC * D     # 512 sharded feature cols per core
EC = E // 128    # 8 e-chunks
TT = S // 128    # 16 token tiles
QCH = S // 512   # 4 query chunks
NB = S // 128    # 16 key blocks

BF16 = ml_dtypes.bfloat16

_CACHE = {}


def _build():
    import concourse.tile as tile
    from concourse import bacc, mybir

    F32 = mybir.dt.float32
    BF = mybir.dt.bfloat16
    AF = mybir.ActivationFunctionType
    ALU = mybir.AluOpType

    nc = bacc.Bacc("TRN2", target_bir_lowering=False, debug=False, num_devices=8)

    xT_d = nc.dram_tensor("xT", [EC, 128, S], BF, kind="ExternalInput")
    wq_d = nc.dram_tensor("wq", [EC, 128, DC], BF, kind="ExternalInput")
    wk_d = nc.dram_tensor("wk", [EC, 128, DC], BF, kind="ExternalInput")
    wv_d = nc.dram_tensor("wv", [EC, 128, DC], BF, kind="ExternalInput")
    wo_d = nc.dram_tensor("wo", [DC // 128, 128, E], BF, kind="ExternalInput")
    bq_d = nc.dram_tensor("bq", [128, 4], F32, kind="ExternalInput")
    bk_d = nc.dram_tensor("bk", [128, 4], F32, kind="ExternalInput")
    bvb_d = nc.dram_tensor("bvb", [128, HPC, 64], BF, kind="ExternalInput")
    mask_d = nc.dram_tensor("mask", [128, 128], BF, kind="ExternalInput")
    out_d = nc.dram_tensor("out", [TT, 128, E], F32, kind="ExternalOutput")

    with tile.TileContext(nc) as tc:
        with tc.tile_pool(name="const", bufs=1) as cp, \
             tc.tile_pool(name="expp", bufs=1) as expp, \
             tc.tile_pool(name="work", bufs=2) as wp, \
             tc.tile_pool(name="unp", bufs=1) as unp, \
             tc.tile_pool(name="ps_s", bufs=2, space="PSUM") as ps_s, \
             tc.tile_pool(name="ps_av", bufs=2, space="PSUM") as ps_av, \
             tc.tile_pool(name="ps_w", bufs=2, space="PSUM") as ps_w:

            # ---- persistent SBUF tensors + input DMAs ----
            xT = [cp.tile([128, S], BF, tag=f"xT{k}", name=f"xT{k}") for k in range(EC)]
            wq = [cp.tile([128, DC], BF, tag=f"wq{k}", name=f"wq{k}") for k in range(EC)]
            wk = [cp.tile([128, DC], BF, tag=f"wk{k}", name=f"wk{k}") for k in range(EC)]
            wv = [cp.tile([128, DC], BF, tag=f"wv{k}", name=f"wv{k}") for k in range(EC)]
            wo = [cp.tile([128, E], BF, tag=f"wo{k}", name=f"wo{k}") for k in range(DC // 128)]
            # spread input DMAs across the 3 DMA-capable queues
            for k in range(EC):
                nc.sync.dma_start(xT[k][:], xT_d.ap()[k])
                nc.scalar.dma_start(wq[k][:], wq_d.ap()[k])
                nc.gpsimd.dma_start(wk[k][:], wk_d.ap()[k])
                nc.scalar.dma_start(wv[k][:], wv_d.ap()[k])
            for k in range(DC // 128):
                nc.gpsimd.dma_start(wo[k][:], wo_d.ap()[k])
            bq = cp.tile([128, 4], F32, tag="bq", name="bq")
            bk = cp.tile([128, 4], F32, tag="bk", name="bk")
            bvb = cp.tile([128, HPC, 64], BF, tag="bvb", name="bvb")
            mask = cp.tile([128, 128], BF, tag="mask", name="mask")
            nc.gpsimd.dma_start(bq[:], bq_d.ap())
            nc.gpsimd.dma_start(bk[:], bk_d.ap())
            nc.gpsimd.dma_start(bvb[:], bvb_d.ap())
            nc.gpsimd.dma_start(mask[:], mask_d.ap())
            ones = cp.tile([128, 64], BF, tag="ones", name="ones")
            nc.any.memset(ones[:], 1.0)
            # softmax-denominator batch tiles: heads at partitions 0/32/64/96
            # (DVE partition bases must be 32-aligned); junk rows preset to
            # 1.0 so the batched reciprocal stays finite
            den = [cp.tile([128, 512], BF, tag=f"den{j}", name=f"den{j}")
                   for j in range(2)]
            rcb = [cp.tile([128, 512], BF, tag=f"rcb{j}", name=f"rcb{j}")
                   for j in range(2)]
            nc.any.memset(den[0][:], 1.0)
            nc.any.memset(den[1][:], 1.0)
            # HAM warmup: the PE is idle for the first ~14us (input DMA);
            # a junk-matmul burst flips the clock gate to 8/8 before the
            # first real projection lands
            wps = ps_w.tile([128, 512], F32, tag="psw", name="psw")
            for _ in range(160):
                nc.tensor.matmul(wps[0:64, 0:64], ones[:, 0:64],
                                 ones[:, 0:64], start=True, stop=True)

            QT = [cp.tile([128, S], BF, tag=f"QT{t}", name=f"QT{t}") for t in range(4)]
            KT = [cp.tile([128, S], BF, tag=f"KT{t}", name=f"KT{t}") for t in range(4)]
            V = [cp.tile([128, HPC, 66], BF, tag=f"V{s}", name=f"V{s}") for s in range(TT)]
            AOT = [cp.tile([128, S], BF, tag=f"AOT{t}", name=f"AOT{t}") for t in range(4)]

            filler = []

            def proj_group(w_sb, b_sb, dst, t, qc):
                def emit():
                    ps = ps_w.tile([128, 512], F32, tag="psw", name="psw")
                    for k in range(EC):
                        nc.tensor.matmul(
                            ps[:],
                            w_sb[k][:, t * 128:(t + 1) * 128],
                            xT[k][:, qc * 512:(qc + 1) * 512],
                            start=(k == 0), stop=(k == EC - 1))
                    nc.vector.tensor_scalar(
                        dst[t][:, qc * 512:(qc + 1) * 512], ps[:],
                        b_sb[:, t:t + 1], None, ALU.add)
                return emit

            def v_group(s):
                def emit():
                    ps = ps_w.tile([128, 512], F32, tag="psw", name="psw")
                    for k in range(EC):
                        nc.tensor.matmul(
                            ps[:],
                            xT[k][:, s * 128:(s + 1) * 128],
                            wv[k][:],
                            start=(k == 0), stop=(k == EC - 1))
                    # V-bias folded in here: softmax weights sum to 1, so
                    # adding bv to V equals adding bv to the attention output
                    nc.vector.tensor_tensor(
                        V[s][:, :, 0:64],
                        ps[:].rearrange("p (h d) -> p h d", d=64),
                        bvb[:], ALU.add)
                    nc.any.memset(V[s][:, :, 64:65], 1.0)
                return emit

            def d_group(s):
                def emit():
                    osb = wp.tile([128, E], F32, tag="osb", name="osb")
                    for n in range(2):
                        ps = ps_w.tile([128, 512], F32, tag="psw", name="psw")
                        for k in range(DC // 128):
                            nc.tensor.matmul(
                                ps[:],
                                AOT[k][:, s * 128:(s + 1) * 128],
                                wo[k][:, n * 512:(n + 1) * 512],
                                start=(k == 0), stop=(k == DC // 128 - 1))
                        nc.vector.tensor_copy(out=osb[:, n * 512:(n + 1) * 512],
                                              in_=ps[:])
                    nc.sync.dma_start(out_d.ap()[s], osb[:])
                return emit

            # up-front: pair-0 projections + first V tiles
            for t in range(4):
                for qc in range(QCH):
                    if t == 0:
                        proj_group(wq, bq, QT, t, qc)()
                        proj_group(wk, bk, KT, t, qc)()
                    else:
                        filler.append(("qkt", t, proj_group(wq, bq, QT, t, qc)))
                        filler.append(("qkt", t, proj_group(wk, bk, KT, t, qc)))
            for s in range(TT):
                if s < 4:
                    v_group(s)()
                else:
                    filler.append(("v", s, v_group(s)))

            def emit_filler_until(pred_drop):
                keep = []
                for item in filler:
                    if pred_drop(item):
                        item[2]()
                    else:
                        keep.append(item)
                filler[:] = keep

            def emit_some_filler(n):
                for _ in range(min(n, len(filler))):
                    filler.pop(0)[2]()

            # ---- attention, head-pair interleaved, qc-outer ----
            # qc=3 (the longest phase) runs first so it absorbs the
            # projection fillers; V tiles are forced in per-round just
            # before the key block that consumes them.
            un = {}
            for qc in (3, 2, 1, 0):
                nkb = 4 * qc + 4
                for hp in range(4):
                    emit_filler_until(lambda it: it[0] == "qkt" and it[1] <= hp)
                    hA, hB = 2 * hp, 2 * hp + 1
                    pav = {}
                    expT = {}
                    pav[hA] = ps_av.tile([128, 512], F32, tag="pav", name="pav")
                    pav[hB] = ps_av.tile([128, 512], F32, tag="pav", name="pav")
                    expT[hA] = expp.tile([128, NB, 512], BF, tag="expTA",
                                         name="expTA")
                    expT[hB] = expp.tile([128, NB, 512], BF, tag="expTB",
                                         name="expTB")

                    def emit_av(kbs_offs):
                        for h in (hA, hB):
                            for kb, off in kbs_offs:
                                nc.tensor.matmul(
                                    pav[h][0:65, off:512],
                                    V[kb][:, h, 0:65],
                                    expT[h][:, kb, off:512],
                                    start=(kb == 0), stop=(kb == nkb - 1))

                    for s0 in range(0, nkb, 2):
                        kbs = list(range(s0, min(s0 + 2, nkb)))
                        emit_filler_until(
                            lambda it: it[0] == "v" and it[1] <= kbs[-1])
                        pss = {h: ps_s.tile([128, 2, 512], F32, tag="pss",
                                            name="pss")
                               for h in (hA, hB)}
                        offs = {}
                        for i, kb in enumerate(kbs):
                            dj = kb - 4 * qc
                            off = 128 * dj if dj > 0 else 0
                            offs[kb] = off
                            for h, r in ((hA, 0), (hB, 64)):
                                nc.tensor.matmul(
                                    pss[h][:, i, off:512],
                                    KT[hp][r:r + 64, kb * 128:(kb + 1) * 128],
                                    QT[hp][r:r + 64,
                                           qc * 512 + off:(qc + 1) * 512],
                                    start=True, stop=True)
                        for h in (hB, hA):
                            # one full-width exp per round (junk columns
                            # below the causal off are never read by AV);
                            # emitted B-first so the first-queued score MM
                            # of the next round frees last (pairs issue
                            # together); diagonal masking on GpSimd
                            nc.scalar.activation(
                                expT[h][:, s0:s0 + len(kbs), :],
                                pss[h][:, 0:len(kbs), :],
                                AF.Exp, scale=0.125)
                            for i, kb in enumerate(kbs):
                                dj = kb - 4 * qc
                                if dj >= 0:
                                    off = offs[kb]
                                    nc.gpsimd.tensor_tensor(
                                        expT[h][:, kb, off:off + 128],
                                        expT[h][:, kb, off:off + 128],
                                        mask[:], ALU.mult)
                        emit_av([(kb, offs[kb]) for kb in kbs])
                        emit_some_filler(1)
                    # evacuate PSUM fast (one bf16 copy) so the next
                    # head-pair's AV can reuse the bank; normalization is
                    # deferred and batched at end-of-qc
                    for h, r in ((hA, 0), (hB, 64)):
                        h01 = 0 if h == hA else 1
                        i = 2 * hp + h01
                        un[i] = unp.tile([65, 512], BF, tag=f"un{i}",
                                         name=f"un{i}")
                        nc.vector.tensor_copy(out=un[i][:],
                                              in_=pav[h][0:65, :])
                # deferred batched normalization for all 8 heads of this qc
                for j in range(2):
                    for m in range(4):
                        i = 4 * j + m
                        nc.vector.tensor_copy(out=den[j][32 * m:32 * m + 1, :],
                                              in_=un[i][64:65, :])
                    with nc.allow_low_precision("softmax denom"):
                        nc.vector.reciprocal(out=rcb[j][:], in_=den[j][:])
                for i2 in range(4):
                    iA, iB = 2 * i2, 2 * i2 + 1
                    psb = ps_w.tile([128, 512], F32, tag="psw", name="psw")
                    for i, base in ((iA, 0), (iB, 64)):
                        j, m = divmod(i, 4)
                        nc.tensor.matmul(psb[base:base + 64, :],
                                         ones[32 * m:32 * m + 1, :],
                                         rcb[j][32 * m:32 * m + 1, :],
                                         start=True, stop=True,
                                         tile_position=(32 * m, base))
                    for i, base in ((iA, 0), (iB, 64)):
                        hp_i, hh = divmod(i, 2)
                        dst = AOT[hp_i][64 * hh:64 * hh + 64,
                                        qc * 512:(qc + 1) * 512]
                        nc.vector.tensor_tensor(dst, un[i][0:64, :],
                                                psb[base:base + 64, :],
                                                ALU.mult)
                # out-projection for this qc becomes filler for the next
                for s in range(qc * 4, qc * 4 + 4):
                    filler.append(("d", s, d_group(s)))
            emit_filler_until(lambda it: True)

    nc.compile()
    return nc


def _get_nc():
    if "nc" not in _CACHE:
        _CACHE["nc"] = _build()
    return _CACHE["nc"]


def _shard_inputs(x, Wq, bq, Wk, bk, Wv, bv, Wo):
    """Build the 8 per-core input maps (host-side shard/cast/transpose)."""
    x = np.asarray(x, np.float32)
    mask = np.triu(np.ones((128, 128), np.float32)).astype(BF16)  # [k, q] q>=k
    in_maps = []
    for c in range(8):
        b, hg = divmod(c, 2)
        dc = slice(hg * DC, (hg + 1) * DC)
        xT = np.ascontiguousarray(x[b].T).astype(BF16).reshape(EC, 128, S)
        wq_c = np.ascontiguousarray(Wq[:, dc]).astype(BF16).reshape(EC, 128, DC)
        wk_c = np.ascontiguousarray(Wk[:, dc]).astype(BF16).reshape(EC, 128, DC)
        wv_c = np.ascontiguousarray(Wv[:, dc]).astype(BF16).reshape(EC, 128, DC)
        wo_c = np.ascontiguousarray(Wo[dc, :]).astype(BF16).reshape(DC // 128, 128, E)
        bq_c = np.ascontiguousarray(np.asarray(bq[dc], np.float32).reshape(4, 128).T)
        bk_c = np.ascontiguousarray(np.asarray(bk[dc], np.float32).reshape(4, 128).T)
        bvb_c = np.ascontiguousarray(np.broadcast_to(
            np.asarray(bv[dc], np.float32).reshape(1, HPC, 64),
            (128, HPC, 64))).astype(BF16)
        in_maps.append({
            "xT": xT, "wq": wq_c, "wk": wk_c, "wv": wv_c, "wo": wo_c,
            "bq": bq_c, "bk": bk_c, "bvb": bvb_c, "mask": mask,
        })
    return in_maps


def kernel(x, Wq, bq, Wk, bk, Wv, bv, Wo, bo):
    from concourse.bass_utils import run_bass_kernel_spmd

    nc = _get_nc()
    in_maps = _shard_inputs(x, Wq, bq, Wk, bk, Wv, bv, Wo)
    res = run_bass_kernel_spmd(nc, in_maps, core_ids=list(range(8)))
    bo = np.asarray(bo, np.float32)
    out = np.empty((B, S, E), np.float32)
    for b in range(B):
        p0 = res.results[2 * b]["out"].reshape(S, E)
        p1 = res.results[2 * b + 1]["out"].reshape(S, E)
        out[b] = p0 + p1 + bo
    return out


# revision 28
# speedup vs baseline: 1.0111x; 1.0077x over previous
"""Multi-head self-attention (B=4, S=2048, E=1024, H=16, causal) on 8 NeuronCores.

Sharding (Megatron-style, per hint): data-parallel over B (4) x tensor-parallel
over heads (2 groups of 8 heads). Core c handles batch c//2 with head-group
c%2: Wq/Wk/Wv sharded column-wise, Wo row-wise. Each core emits a partial
out-projection [S, E]; the host sums each pair of partials (the "all-reduce")
and adds bo.

Per-core kernel (all matmuls bf16, fp32 accumulation):
  - host supplies x[b].T so Q^T,K^T ([d,s]) and V ([s,d]) come straight off
    the projections with no on-chip transposes
  - scores computed transposed (S^T = K Q^T, [keys, queries]) with causal
    block-skipping; the two heads of a pair issue as row-tiled (K=64)
    matmuls that can run concurrently; exp on ScalarE with fused 1/sqrt(D)
    scale reads PSUM directly and writes bf16; diagonal-block masking runs
    on GpSimd to keep the DVE queue clear
  - softmax denominator comes free from a ones-column appended to V in the
    attn @ V matmul; pav is evacuated to SBUF immediately (frees the PSUM
    bank) and normalization is deferred to end-of-qc and batched:
    denominators gathered at 32-aligned partitions, one free-dim-bound
    reciprocal per 4 heads, col-tiled rank-1 PE broadcasts, one vector
    multiply per head
  - V-bias is folded into V at projection time (softmax weights sum to 1)
  - a junk-matmul warmup burst spans the input-DMA window so the HAM clock
    gate is at 8/8 when the first projection lands
"""

import numpy as np
import ml_dtypes

B, S, E, H, D = 4, 2048, 1024, 16, 64
HPC = 8          # heads per core
DC = HPC * D     # 512 sharded feature cols per core
EC = E // 128    # 8 e-chunks
TT = S // 128    # 16 token tiles
QCH = S // 512   # 4 query chunks
NB = S // 128    # 16 key blocks

BF16 = ml_dtypes.bfloat16

_CACHE = {}


def _build():
    import concourse.tile as tile
    from concourse import bacc, mybir

    F32 = mybir.dt.float32
    BF = mybir.dt.bfloat16
    AF = mybir.ActivationFunctionType
    ALU = mybir.AluOpType

    nc = bacc.Bacc("TRN2", target_bir_lowering=False, debug=False, num_devices=8)

    xT_d = nc.dram_tensor("xT", [EC, 128, S], BF, kind="ExternalInput")
    wq_d = nc.dram_tensor("wq", [EC, 128, DC], BF, kind="ExternalInput")
    wk_d = nc.dram_tensor("wk", [EC, 128, DC], BF, kind="ExternalInput")
    wv_d = nc.dram_tensor("wv", [EC, 128, DC], BF, kind="ExternalInput")
    wo_d = nc.dram_tensor("wo", [DC // 128, 128, E], BF, kind="ExternalInput")
    bq_d = nc.dram_tensor("bq", [128, 4], F32, kind="ExternalInput")
    bk_d = nc.dram_tensor("bk", [128, 4], F32, kind="ExternalInput")
    bvb_d = nc.dram_tensor("bvb", [128, HPC, 64], BF, kind="ExternalInput")
    mask_d = nc.dram_tensor("mask", [128, 128], BF, kind="ExternalInput")
    out_d = nc.dram_tensor("out", [TT, 128, E], BF, kind="ExternalOutput")

    with tile.TileContext(nc) as tc:
        with tc.tile_pool(name="const", bufs=1) as cp, \
             tc.tile_pool(name="expp", bufs=1) as expp, \
             tc.tile_pool(name="work", bufs=2) as wp, \
             tc.tile_pool(name="unp", bufs=1) as unp, \
             tc.tile_pool(name="ps_s", bufs=2, space="PSUM") as ps_s, \
             tc.tile_pool(name="ps_av", bufs=2, space="PSUM") as ps_av, \
             tc.tile_pool(name="ps_w", bufs=2, space="PSUM") as ps_w:

            # ---- persistent SBUF tensors + input DMAs ----
            xT = [cp.tile([128, S], BF, tag=f"xT{k}", name=f"xT{k}") for k in range(EC)]
            wq = [cp.tile([128, DC], BF, tag=f"wq{k}", name=f"wq{k}") for k in range(EC)]
            wk = [cp.tile([128, DC], BF, tag=f"wk{k}", name=f"wk{k}") for k in range(EC)]
            wv = [cp.tile([128, DC], BF, tag=f"wv{k}", name=f"wv{k}") for k in range(EC)]
            wo = [cp.tile([128, E], BF, tag=f"wo{k}", name=f"wo{k}") for k in range(DC // 128)]
            # spread input DMAs across the 3 DMA-capable queues
            for k in range(EC):
                nc.sync.dma_start(xT[k][:], xT_d.ap()[k])
                nc.scalar.dma_start(wq[k][:], wq_d.ap()[k])
                nc.gpsimd.dma_start(wk[k][:], wk_d.ap()[k])
                nc.scalar.dma_start(wv[k][:], wv_d.ap()[k])
            for k in range(DC // 128):
                nc.gpsimd.dma_start(wo[k][:], wo_d.ap()[k])
            bq = cp.tile([128, 4], F32, tag="bq", name="bq")
            bk = cp.tile([128, 4], F32, tag="bk", name="bk")
            bvb = cp.tile([128, HPC, 64], BF, tag="bvb", name="bvb")
            mask = cp.tile([128, 128], BF, tag="mask", name="mask")
            nc.gpsimd.dma_start(bq[:], bq_d.ap())
            nc.gpsimd.dma_start(bk[:], bk_d.ap())
            nc.gpsimd.dma_start(bvb[:], bvb_d.ap())
            nc.gpsimd.dma_start(mask[:], mask_d.ap())
            ones = cp.tile([128, 64], BF, tag="ones", name="ones")
            nc.any.memset(ones[:], 1.0)
            # softmax-denominator batch tiles: heads at partitions 0/32/64/96
            # (DVE partition bases must be 32-aligned); junk rows preset to
            # 1.0 so the batched reciprocal stays finite
            den = [cp.tile([128, 512], BF, tag=f"den{j}", name=f"den{j}")
                   for j in range(2)]
            rcb = [cp.tile([128, 512], BF, tag=f"rcb{j}", name=f"rcb{j}")
                   for j in range(2)]
            nc.any.memset(den[0][:], 1.0)
            nc.any.memset(den[1][:], 1.0)
            # HAM warmup: the PE is idle for the first ~14us (input DMA);
            # a junk-matmul burst flips the clock gate to 8/8 before the
            # first real projection lands
            wps = ps_w.tile([128, 512], F32, tag="psw", name="psw")
            for _ in range(160):
                nc.tensor.matmul(wps[0:64, 0:64], ones[:, 0:64],
                                 ones[:, 0:64], start=True, stop=True)

            QT = [cp.tile([128, S], BF, tag=f"QT{t}", name=f"QT{t}") for t in range(4)]
            KT = [cp.tile([128, S], BF, tag=f"KT{t}", name=f"KT{t}") for t in range(4)]
            V = [cp.tile([128, HPC, 66], BF, tag=f"V{s}", name=f"V{s}") for s in range(TT)]
            AOT = [cp.tile([128, S], BF, tag=f"AOT{t}", name=f"AOT{t}") for t in range(4)]

            filler = []

            def proj_group(w_sb, b_sb, dst, t, qc):
                def emit():
                    ps = ps_w.tile([128, 512], F32, tag="psw", name="psw")
                    for k in range(EC):
                        nc.tensor.matmul(
                            ps[:],
                            w_sb[k][:, t * 128:(t + 1) * 128],
                            xT[k][:, qc * 512:(qc + 1) * 512],
                            start=(k == 0), stop=(k == EC - 1))
                    nc.vector.tensor_scalar(
                        dst[t][:, qc * 512:(qc + 1) * 512], ps[:],
                        b_sb[:, t:t + 1], None, ALU.add)
                return emit

            def v_group(s):
                def emit():
                    ps = ps_w.tile([128, 512], F32, tag="psw", name="psw")
                    for k in range(EC):
                        nc.tensor.matmul(
                            ps[:],
                            xT[k][:, s * 128:(s + 1) * 128],
                            wv[k][:],
                            start=(k == 0), stop=(k == EC - 1))
                    # V-bias folded in here: softmax weights sum to 1, so
                    # adding bv to V equals adding bv to the attention output
                    nc.vector.tensor_tensor(
                        V[s][:, :, 0:64],
                        ps[:].rearrange("p (h d) -> p h d", d=64),
                        bvb[:], ALU.add)
                    nc.any.memset(V[s][:, :, 64:65], 1.0)
                return emit

            def d_group(s):
                def emit():
                    osb = wp.tile([128, E], BF, tag="osb", name="osb")
                    for n in range(2):
                        ps = ps_w.tile([128, 512], F32, tag="psw", name="psw")
                        for k in range(DC // 128):
                            nc.tensor.matmul(
                                ps[:],
                                AOT[k][:, s * 128:(s + 1) * 128],
                                wo[k][:, n * 512:(n + 1) * 512],
                                start=(k == 0), stop=(k == DC // 128 - 1))
                        nc.vector.tensor_copy(out=osb[:, n * 512:(n + 1) * 512],
                                              in_=ps[:])
                    nc.sync.dma_start(out_d.ap()[s], osb[:])
                return emit

            # up-front: pair-0 projections + first V tiles
            for t in range(4):
                for qc in range(QCH):
                    if t == 0:
                        proj_group(wq, bq, QT, t, qc)()
                        proj_group(wk, bk, KT, t, qc)()
                    else:
                        filler.append(("qkt", t, proj_group(wq, bq, QT, t, qc)))
                        filler.append(("qkt", t, proj_group(wk, bk, KT, t, qc)))
            for s in range(TT):
                if s < 4:
                    v_group(s)()
                else:
                    filler.append(("v", s, v_group(s)))

            def emit_filler_until(pred_drop):
                keep = []
                for item in filler:
                    if pred_drop(item):
                        item[2]()
                    else:
                        keep.append(item)
                filler[:] = keep

            def emit_some_filler(n):
                for _ in range(min(n, len(filler))):
                    filler.pop(0)[2]()

            # ---- attention, head-pair interleaved, qc-outer ----
            # qc=3 (the longest phase) runs first so it absorbs the
            # projection fillers; V tiles are forced in per-round just
            # before the key block that consumes them.
            un = {}
            for qc in (3, 2, 1, 0):
                nkb = 4 * qc + 4
                for hp in range(4):
                    emit_filler_until(lambda it: it[0] == "qkt" and it[1] <= hp)
                    hA, hB = 2 * hp, 2 * hp + 1
                    pav = {}
                    expT = {}
                    pav[hA] = ps_av.tile([128, 512], F32, tag="pav", name="pav")
                    pav[hB] = ps_av.tile([128, 512], F32, tag="pav", name="pav")
                    expT[hA] = expp.tile([128, NB, 512], BF, tag="expTA",
                                         name="expTA")
                    expT[hB] = expp.tile([128, NB, 512], BF, tag="expTB",
                                         name="expTB")

                    def emit_av(kbs_offs):
                        for h in (hA, hB):
                            for kb, off in kbs_offs:
                                nc.tensor.matmul(
                                    pav[h][0:65, off:512],
                                    V[kb][:, h, 0:65],
                                    expT[h][:, kb, off:512],
                                    start=(kb == 0), stop=(kb == nkb - 1))

                    for s0 in range(0, nkb, 2):
                        kbs = list(range(s0, min(s0 + 2, nkb)))
                        emit_filler_until(
                            lambda it: it[0] == "v" and it[1] <= kbs[-1])
                        pss = {h: ps_s.tile([128, 2, 512], F32, tag="pss",
                                            name="pss")
                               for h in (hA, hB)}
                        offs = {}
                        for i, kb in enumerate(kbs):
                            dj = kb - 4 * qc
                            off = 128 * dj if dj > 0 else 0
                            offs[kb] = off
                            for h, r in ((hA, 0), (hB, 64)):
                                nc.tensor.matmul(
                                    pss[h][:, i, off:512],
                                    KT[hp][r:r + 64, kb * 128:(kb + 1) * 128],
                                    QT[hp][r:r + 64,
                                           qc * 512 + off:(qc + 1) * 512],
                                    start=True, stop=True)
                        for h in (hB, hA):
                            # one full-width exp per round (junk columns
                            # below the causal off are never read by AV);
                            # emitted B-first so the first-queued score MM
                            # of the next round frees last (pairs issue
                            # together); diagonal masking on GpSimd
                            nc.scalar.activation(
                                expT[h][:, s0:s0 + len(kbs), :],
                                pss[h][:, 0:len(kbs), :],
                                AF.Exp, scale=0.125)
                            for i, kb in enumerate(kbs):
                                dj = kb - 4 * qc
                                if dj >= 0:
                                    off = offs[kb]
                                    nc.gpsimd.tensor_tensor(
                                        expT[h][:, kb, off:off + 128],
                                        expT[h][:, kb, off:off + 128],
                                        mask[:], ALU.mult)
                        emit_av([(kb, offs[kb]) for kb in kbs])
                        emit_some_filler(1)
                    # evacuate PSUM fast (one bf16 copy) so the next
                    # head-pair's AV can reuse the bank; normalization is
                    # deferred and batched at end-of-qc
                    for h, r in ((hA, 0), (hB, 64)):
                        h01 = 0 if h == hA else 1
                        i = 2 * hp + h01
                        un[i] = unp.tile([65, 512], BF, tag=f"un{i}",
                                         name=f"un{i}")
                        nc.vector.tensor_copy(out=un[i][:],
                                              in_=pav[h][0:65, :])
                # deferred batched normalization for all 8 heads of this qc
                for j in range(2):
                    for m in range(4):
                        i = 4 * j + m
                        nc.vector.tensor_copy(out=den[j][32 * m:32 * m + 1, :],
                                              in_=un[i][64:65, :])
                    with nc.allow_low_precision("softmax denom"):
                        nc.vector.reciprocal(out=rcb[j][:], in_=den[j][:])
                for i2 in range(4):
                    iA, iB = 2 * i2, 2 * i2 + 1
                    psb = ps_w.tile([128, 512], F32, tag="psw", name="psw")
                    for i, base in ((iA, 0), (iB, 64)):
                        j, m = divmod(i, 4)
                        nc.tensor.matmul(psb[base:base + 64, :],
                                         ones[32 * m:32 * m + 1, :],
                                         rcb[j][32 * m:32 * m + 1, :],
                                         start=True, stop=True,
                                         tile_position=(32 * m, base))
                    for i, base in ((iA, 0), (iB, 64)):
                        hp_i, hh = divmod(i, 2)
                        dst = AOT[hp_i][64 * hh:64 * hh + 64,
                                        qc * 512:(qc + 1) * 512]
                        nc.vector.tensor_tensor(dst, un[i][0:64, :],
                                                psb[base:base + 64, :],
                                                ALU.mult)
                # out-projection for this qc becomes filler for the next
                for s in range(qc * 4, qc * 4 + 4):
                    filler.append(("d", s, d_group(s)))
            emit_filler_until(lambda it: True)

    nc.compile()
    return nc


def _get_nc():
    if "nc" not in _CACHE:
        _CACHE["nc"] = _build()
    return _CACHE["nc"]


def _shard_inputs(x, Wq, bq, Wk, bk, Wv, bv, Wo):
    """Build the 8 per-core input maps (host-side shard/cast/transpose)."""
    x = np.asarray(x, np.float32)
    mask = np.triu(np.ones((128, 128), np.float32)).astype(BF16)  # [k, q] q>=k
    in_maps = []
    for c in range(8):
        b, hg = divmod(c, 2)
        dc = slice(hg * DC, (hg + 1) * DC)
        xT = np.ascontiguousarray(x[b].T).astype(BF16).reshape(EC, 128, S)
        wq_c = np.ascontiguousarray(Wq[:, dc]).astype(BF16).reshape(EC, 128, DC)
        wk_c = np.ascontiguousarray(Wk[:, dc]).astype(BF16).reshape(EC, 128, DC)
        wv_c = np.ascontiguousarray(Wv[:, dc]).astype(BF16).reshape(EC, 128, DC)
        wo_c = np.ascontiguousarray(Wo[dc, :]).astype(BF16).reshape(DC // 128, 128, E)
        bq_c = np.ascontiguousarray(np.asarray(bq[dc], np.float32).reshape(4, 128).T)
        bk_c = np.ascontiguousarray(np.asarray(bk[dc], np.float32).reshape(4, 128).T)
        bvb_c = np.ascontiguousarray(np.broadcast_to(
            np.asarray(bv[dc], np.float32).reshape(1, HPC, 64),
            (128, HPC, 64))).astype(BF16)
        in_maps.append({
            "xT": xT, "wq": wq_c, "wk": wk_c, "wv": wv_c, "wo": wo_c,
            "bq": bq_c, "bk": bk_c, "bvb": bvb_c, "mask": mask,
        })
    return in_maps


def kernel(x, Wq, bq, Wk, bk, Wv, bv, Wo, bo):
    from concourse.bass_utils import run_bass_kernel_spmd

    nc = _get_nc()
    in_maps = _shard_inputs(x, Wq, bq, Wk, bk, Wv, bv, Wo)
    res = run_bass_kernel_spmd(nc, in_maps, core_ids=list(range(8)))
    bo = np.asarray(bo, np.float32)
    out = np.empty((B, S, E), np.float32)
    for b in range(B):
        p0 = res.results[2 * b]["out"].reshape(S, E).astype(np.float32)
        p1 = res.results[2 * b + 1]["out"].reshape(S, E).astype(np.float32)
        out[b] = p0 + p1 + bo
    return out


# revision 30
# speedup vs baseline: 1.0131x; 1.0020x over previous
"""Multi-head self-attention (B=4, S=2048, E=1024, H=16, causal) on 8 NeuronCores.

Sharding (Megatron-style, per hint): data-parallel over B (4) x tensor-parallel
over heads (2 groups of 8 heads). Core c handles batch c//2 with head-group
c%2: Wq/Wk/Wv sharded column-wise, Wo row-wise. Each core emits a partial
out-projection [S, E]; the host sums each pair of partials (the "all-reduce")
and adds bo.

Per-core kernel (all matmuls bf16, fp32 accumulation):
  - host supplies x[b].T so Q^T,K^T ([d,s]) and V ([s,d]) come straight off
    the projections with no on-chip transposes
  - scores computed transposed (S^T = K Q^T, [keys, queries]) with causal
    block-skipping; the two heads of a pair issue as row-tiled (K=64)
    matmuls that can run concurrently; exp on ScalarE with fused 1/sqrt(D)
    scale reads PSUM directly and writes bf16; diagonal-block masking runs
    on GpSimd to keep the DVE queue clear
  - softmax denominator comes free from a ones-column appended to V in the
    attn @ V matmul; pav is evacuated to SBUF immediately (frees the PSUM
    bank) and normalization is deferred to end-of-qc and batched:
    denominators gathered at 32-aligned partitions, one free-dim-bound
    reciprocal per 4 heads, col-tiled rank-1 PE broadcasts, one vector
    multiply per head
  - V-bias is folded into V at projection time (softmax weights sum to 1)
  - a junk-matmul warmup burst spans the input-DMA window so the HAM clock
    gate is at 8/8 when the first projection lands
"""

import numpy as np
import ml_dtypes

B, S, E, H, D = 4, 2048, 1024, 16, 64
HPC = 8          # heads per core
DC = HPC * D     # 512 sharded feature cols per core
EC = E // 128    # 8 e-chunks
TT = S // 128    # 16 token tiles
QCH = S // 512   # 4 query chunks
NB = S // 128    # 16 key blocks

BF16 = ml_dtypes.bfloat16

_CACHE = {}


def _build():
    import concourse.tile as tile
    from concourse import bacc, mybir

    F32 = mybir.dt.float32
    BF = mybir.dt.bfloat16
    AF = mybir.ActivationFunctionType
    ALU = mybir.AluOpType

    nc = bacc.Bacc("TRN2", target_bir_lowering=False, debug=False, num_devices=8)

    xT_d = nc.dram_tensor("xT", [EC, 128, S], BF, kind="ExternalInput")
    wq_d = nc.dram_tensor("wq", [EC, 128, DC], BF, kind="ExternalInput")
    wk_d = nc.dram_tensor("wk", [EC, 128, DC], BF, kind="ExternalInput")
    wv_d = nc.dram_tensor("wv", [EC, 128, DC], BF, kind="ExternalInput")
    wo_d = nc.dram_tensor("wo", [DC // 128, 128, E], BF, kind="ExternalInput")
    bq_d = nc.dram_tensor("bq", [128, 4], F32, kind="ExternalInput")
    bk_d = nc.dram_tensor("bk", [128, 4], F32, kind="ExternalInput")
    bvb_d = nc.dram_tensor("bvb", [128, HPC, 64], BF, kind="ExternalInput")
    mask_d = nc.dram_tensor("mask", [128, 128], BF, kind="ExternalInput")
    out_d = nc.dram_tensor("out", [TT, 128, E], BF, kind="ExternalOutput")

    with tile.TileContext(nc) as tc:
        with tc.tile_pool(name="const", bufs=1) as cp, \
             tc.tile_pool(name="expp", bufs=1) as expp, \
             tc.tile_pool(name="work", bufs=2) as wp, \
             tc.tile_pool(name="unp", bufs=1) as unp, \
             tc.tile_pool(name="ps_s", bufs=2, space="PSUM") as ps_s, \
             tc.tile_pool(name="ps_av", bufs=2, space="PSUM") as ps_av, \
             tc.tile_pool(name="ps_w", bufs=2, space="PSUM") as ps_w:

            # ---- persistent SBUF tensors + input DMAs ----
            xT = [cp.tile([128, S], BF, tag=f"xT{k}", name=f"xT{k}") for k in range(EC)]
            wq = [cp.tile([128, DC], BF, tag=f"wq{k}", name=f"wq{k}") for k in range(EC)]
            wk = [cp.tile([128, DC], BF, tag=f"wk{k}", name=f"wk{k}") for k in range(EC)]
            wv = [cp.tile([128, DC], BF, tag=f"wv{k}", name=f"wv{k}") for k in range(EC)]
            wo = [cp.tile([128, E], BF, tag=f"wo{k}", name=f"wo{k}") for k in range(DC // 128)]
            # spread input DMAs across the 3 DMA-capable queues
            for k in range(EC):
                nc.sync.dma_start(xT[k][:], xT_d.ap()[k])
                nc.scalar.dma_start(wq[k][:], wq_d.ap()[k])
                nc.gpsimd.dma_start(wk[k][:], wk_d.ap()[k])
                nc.scalar.dma_start(wv[k][:], wv_d.ap()[k])
            for k in range(DC // 128):
                nc.gpsimd.dma_start(wo[k][:], wo_d.ap()[k])
            bq = cp.tile([128, 4], F32, tag="bq", name="bq")
            bk = cp.tile([128, 4], F32, tag="bk", name="bk")
            bvb = cp.tile([128, HPC, 64], BF, tag="bvb", name="bvb")
            mask = cp.tile([128, 128], BF, tag="mask", name="mask")
            nc.gpsimd.dma_start(bq[:], bq_d.ap())
            nc.gpsimd.dma_start(bk[:], bk_d.ap())
            nc.gpsimd.dma_start(bvb[:], bvb_d.ap())
            nc.gpsimd.dma_start(mask[:], mask_d.ap())
            ones = cp.tile([128, 64], BF, tag="ones", name="ones")
            nc.any.memset(ones[:], 1.0)
            # softmax-denominator batch tiles: heads at partitions 0/32/64/96
            # (DVE partition bases must be 32-aligned); junk rows preset to
            # 1.0 so the batched reciprocal stays finite
            den = [cp.tile([128, 512], BF, tag=f"den{j}", name=f"den{j}")
                   for j in range(2)]
            rcb = [cp.tile([128, 512], BF, tag=f"rcb{j}", name=f"rcb{j}")
                   for j in range(2)]
            nc.any.memset(den[0][:], 1.0)
            nc.any.memset(den[1][:], 1.0)
            # HAM warmup: the PE is idle for the first ~14us (input DMA);
            # a junk-matmul burst flips the clock gate to 8/8 before the
            # first real projection lands
            wps = ps_w.tile([128, 512], F32, tag="psw", name="psw")
            for _ in range(160):
                nc.tensor.matmul(wps[0:64, 0:64], ones[:, 0:64],
                                 ones[:, 0:64], start=True, stop=True)

            QT = [cp.tile([128, S], BF, tag=f"QT{t}", name=f"QT{t}") for t in range(4)]
            KT = [cp.tile([128, S], BF, tag=f"KT{t}", name=f"KT{t}") for t in range(4)]
            V = [cp.tile([128, HPC, 66], BF, tag=f"V{s}", name=f"V{s}") for s in range(TT)]
            AOT = [cp.tile([128, S], BF, tag=f"AOT{t}", name=f"AOT{t}") for t in range(4)]

            filler = []

            def proj_group(w_sb, b_sb, dst, t, qc):
                def emit():
                    ps = ps_w.tile([128, 512], F32, tag="psw", name="psw")
                    for k in range(EC):
                        nc.tensor.matmul(
                            ps[:],
                            w_sb[k][:, t * 128:(t + 1) * 128],
                            xT[k][:, qc * 512:(qc + 1) * 512],
                            start=(k == 0), stop=(k == EC - 1))
                    nc.vector.tensor_scalar(
                        dst[t][:, qc * 512:(qc + 1) * 512], ps[:],
                        b_sb[:, t:t + 1], None, ALU.add)
                return emit

            def v_group(s):
                def emit():
                    ps = ps_w.tile([128, 512], F32, tag="psw", name="psw")
                    for k in range(EC):
                        nc.tensor.matmul(
                            ps[:],
                            xT[k][:, s * 128:(s + 1) * 128],
                            wv[k][:],
                            start=(k == 0), stop=(k == EC - 1))
                    # V-bias folded in here: softmax weights sum to 1, so
                    # adding bv to V equals adding bv to the attention output
                    nc.vector.tensor_tensor(
                        V[s][:, :, 0:64],
                        ps[:].rearrange("p (h d) -> p h d", d=64),
                        bvb[:], ALU.add)
                    nc.any.memset(V[s][:, :, 64:65], 1.0)
                return emit

            def d_group(s):
                def emit():
                    osb = wp.tile([128, E], BF, tag="osb", name="osb")
                    for n in range(2):
                        ps = ps_w.tile([128, 512], F32, tag="psw", name="psw")
                        for k in range(DC // 128):
                            nc.tensor.matmul(
                                ps[:],
                                AOT[k][:, s * 128:(s + 1) * 128],
                                wo[k][:, n * 512:(n + 1) * 512],
                                start=(k == 0), stop=(k == DC // 128 - 1))
                        nc.vector.tensor_copy(out=osb[:, n * 512:(n + 1) * 512],
                                              in_=ps[:])
                    nc.sync.dma_start(out_d.ap()[s], osb[:])
                return emit

            # up-front: pair-0 projections + first V tiles
            for t in range(4):
                for qc in range(QCH):
                    if t == 0:
                        proj_group(wq, bq, QT, t, qc)()
                        proj_group(wk, bk, KT, t, qc)()
                    else:
                        filler.append(("qkt", t, proj_group(wq, bq, QT, t, qc)))
                        filler.append(("qkt", t, proj_group(wk, bk, KT, t, qc)))
            for s in range(TT):
                if s < 4:
                    v_group(s)()
                else:
                    filler.append(("v", s, v_group(s)))

            def emit_filler_until(pred_drop):
                keep = []
                for item in filler:
                    if pred_drop(item):
                        item[2]()
                    else:
                        keep.append(item)
                filler[:] = keep

            def emit_some_filler(n):
                for _ in range(min(n, len(filler))):
                    filler.pop(0)[2]()

            # ---- attention, head-pair interleaved, qc-outer ----
            # qc=3 (the longest phase) runs first so it absorbs the
            # projection fillers; V tiles are forced in per-round just
            # before the key block that consumes them.
            un = {}
            for qc in (3, 2, 1, 0):
                nkb = 4 * qc + 4
                for hp in range(4):
                    emit_filler_until(lambda it: it[0] == "qkt" and it[1] <= hp)
                    hA, hB = 2 * hp, 2 * hp + 1
                    pav = {}
                    expT = {}
                    pav[hA] = ps_av.tile([128, 512], F32, tag="pav", name="pav")
                    pav[hB] = ps_av.tile([128, 512], F32, tag="pav", name="pav")
                    expT[hA] = expp.tile([128, NB, 512], BF, tag="expTA",
                                         name="expTA")
                    expT[hB] = expp.tile([128, NB, 512], BF, tag="expTB",
                                         name="expTB")

                    def emit_av(kbs_offs):
                        for h in (hA, hB):
                            for kb, off in kbs_offs:
                                nc.tensor.matmul(
                                    pav[h][0:65, off:512],
                                    V[kb][:, h, 0:65],
                                    expT[h][:, kb, off:512],
                                    start=(kb == 0), stop=(kb == nkb - 1))

                    for s0 in range(0, nkb, 2):
                        kbs = list(range(s0, min(s0 + 2, nkb)))
                        emit_filler_until(
                            lambda it: it[0] == "v" and it[1] <= kbs[-1])
                        pss = {h: ps_s.tile([128, 2, 512], F32, tag="pss",
                                            name="pss")
                               for h in (hA, hB)}
                        offs = {}
                        for i, kb in enumerate(kbs):
                            dj = kb - 4 * qc
                            off = 128 * dj if dj > 0 else 0
                            offs[kb] = off
                            for h, r in ((hA, 0), (hB, 64)):
                                nc.tensor.matmul(
                                    pss[h][:, i, off:512],
                                    KT[hp][r:r + 64, kb * 128:(kb + 1) * 128],
                                    QT[hp][r:r + 64,
                                           qc * 512 + off:(qc + 1) * 512],
                                    start=True, stop=True)
                        for h in (hB, hA):
                            # one full-width exp per round (junk columns
                            # below the causal off are never read by AV);
                            # emitted B-first so the first-queued score MM
                            # of the next round frees last (pairs issue
                            # together); diagonal masking on GpSimd
                            nc.scalar.activation(
                                expT[h][:, s0:s0 + len(kbs), :],
                                pss[h][:, 0:len(kbs), :],
                                AF.Exp, scale=0.125)
                            for i, kb in enumerate(kbs):
                                dj = kb - 4 * qc
                                if dj >= 0:
                                    off = offs[kb]
                                    nc.gpsimd.tensor_tensor(
                                        expT[h][:, kb, off:off + 128],
                                        expT[h][:, kb, off:off + 128],
                                        mask[:], ALU.mult)
                        emit_av([(kb, offs[kb]) for kb in kbs])
                        emit_some_filler(1)
                    # evacuate PSUM fast (one bf16 copy) so the next
                    # head-pair's AV can reuse the bank; normalization is
                    # deferred and batched at end-of-qc
                    for h, r in ((hA, 0), (hB, 64)):
                        h01 = 0 if h == hA else 1
                        i = 2 * hp + h01
                        un[i] = unp.tile([65, 512], BF, tag=f"un{i}",
                                         name=f"un{i}")
                        nc.vector.tensor_copy(out=un[i][:],
                                              in_=pav[h][0:65, :])
                # deferred batched normalization for all 8 heads of this qc
                for j in range(2):
                    for m in range(4):
                        i = 4 * j + m
                        nc.vector.tensor_copy(out=den[j][32 * m:32 * m + 1, :],
                                              in_=un[i][64:65, :])
                    with nc.allow_low_precision("softmax denom"):
                        nc.vector.reciprocal(out=rcb[j][:], in_=den[j][:])
                for i2 in range(4):
                    iA, iB = 2 * i2, 2 * i2 + 1
                    psb = ps_w.tile([128, 512], F32, tag="psw", name="psw")
                    for i, base in ((iA, 0), (iB, 64)):
                        j, m = divmod(i, 4)
                        nc.tensor.matmul(psb[base:base + 64, :],
                                         ones[32 * m:32 * m + 1, :],
                                         rcb[j][32 * m:32 * m + 1, :],
                                         start=True, stop=True,
                                         tile_position=(32 * m, base))
                    for i, base in ((iA, 0), (iB, 64)):
                        hp_i, hh = divmod(i, 2)
                        dst = AOT[hp_i][64 * hh:64 * hh + 64,
                                        qc * 512:(qc + 1) * 512]
                        nc.vector.tensor_tensor(dst, un[i][0:64, :],
                                                psb[base:base + 64, :],
                                                ALU.mult)
                # out-projection for this qc becomes filler for the next
                for s in range(qc * 4, qc * 4 + 4):
                    filler.append(("d", s, d_group(s)))
            emit_filler_until(lambda it: True)

    nc.compile()
    return nc


def _get_nc():
    if "nc" not in _CACHE:
        _CACHE["nc"] = _build()
    return _CACHE["nc"]


def _shard_inputs(x, Wq, bq, Wk, bk, Wv, bv, Wo):
    """Build the 8 per-core input maps (host-side shard/cast/transpose)."""
    x = np.asarray(x, np.float32)
    mask = np.triu(np.ones((128, 128), np.float32)).astype(BF16)  # [k, q] q>=k
    in_maps = []
    for c in range(8):
        b, hg = divmod(c, 2)
        dc = slice(hg * DC, (hg + 1) * DC)
        xT = np.ascontiguousarray(x[b].T).astype(BF16).reshape(EC, 128, S)
        wq_c = np.ascontiguousarray(Wq[:, dc]).astype(BF16).reshape(EC, 128, DC)
        wk_c = np.ascontiguousarray(Wk[:, dc]).astype(BF16).reshape(EC, 128, DC)
        wv_c = np.ascontiguousarray(Wv[:, dc]).astype(BF16).reshape(EC, 128, DC)
        wo_c = np.ascontiguousarray(Wo[dc, :]).astype(BF16).reshape(DC // 128, 128, E)
        bq_c = np.ascontiguousarray(np.asarray(bq[dc], np.float32).reshape(4, 128).T)
        bk_c = np.ascontiguousarray(np.asarray(bk[dc], np.float32).reshape(4, 128).T)
        bvb_c = np.ascontiguousarray(np.broadcast_to(
            np.asarray(bv[dc], np.float32).reshape(1, HPC, 64),
            (128, HPC, 64))).astype(BF16)
        in_maps.append({
            "xT": xT, "wq": wq_c, "wk": wk_c, "wv": wv_c, "wo": wo_c,
            "bq": bq_c, "bk": bk_c, "bvb": bvb_c, "mask": mask,
        })
    return in_maps


def kernel(x, Wq, bq, Wk, bk, Wv, bv, Wo, bo):
    from concourse.bass_utils import run_bass_kernel_spmd

    nc = _get_nc()
    in_maps = _shard_inputs(x, Wq, bq, Wk, bk, Wv, bv, Wo)
    res = run_bass_kernel_spmd(nc, in_maps, core_ids=list(range(8)))
    bo = np.asarray(bo, np.float32)
    out = np.empty((B, S, E), np.float32)
    for b in range(B):
        p0 = res.results[2 * b]["out"].reshape(S, E).astype(np.float32)
        p1 = res.results[2 * b + 1]["out"].reshape(S, E).astype(np.float32)
        out[b] = p0 + p1 + bo
    return out
